# revision 2
# baseline (speedup 1.0000x reference)
"""CPC model (conv encoder + GRU + InfoNCE loss) on 8 TRN2 NeuronCores. v2

 - Data-parallel over batch: each core owns 8 of 64 sequences (72 images).
 - conv1 5x5s2 as bf16 matmuls (im2col, K=75); activation columns are
   image-interleaved (n = r*32 + x*2 + i) so 3x3 tap windows are flat
   [128, (2,) 16, 32] APs over zero-padded [18, 40] planes.
 - Per-resblock precision config: fp8-e4m3 DoubleRow (256-ch contraction
   per pass; rb = 26 MMs) or bf16 (rb = 52 MMs). Activations carry a 16x
   scale (lifts fp8 denormals; biases pre-scaled, no per-op descales).
 - Two image-pairs emitted stage-interleaved so PE always has the sibling
   pair's matmuls while drains complete; drains split ACT/DVE; the global
   avg pool rides on the last drain's accum_out for free.
 - Timestep rows [4..8, 0..3]; AllGather of ztk overlaps conv; GRU step t
   emitted right after row t. Log-softmax tail batched over all 5 k with
   one Exp->Ln table switch.
"""
import os
import sys

import numpy as np
import ml_dtypes

for _p in ("/opt/trn_rl_repo", "/root/.axon_site/_ro/trn_rl_repo"):
    if os.path.isdir(_p) and _p not in sys.path:
        sys.path.insert(0, _p)

import concourse.bacc as bacc  # noqa: E402
import concourse.bass as bass  # noqa: E402
import concourse.mybir as mybir  # noqa: E402
import concourse.tile as tile  # noqa: E402
from concourse.bass_utils import run_bass_kernel_spmd  # noqa: E402

F32 = np.float32
BF16 = ml_dtypes.bfloat16
E4 = ml_dtypes.float8_e4m3
DT = mybir.dt
DRMODE = mybir.MatmulPerfMode.DoubleRow

B, T, C = 64, 9, 3
DIM, HALF, HID, R, K = 512, 256, 256, 2, 5
TCTX = 4
NCORES = 8
NB = B // NCORES           # 8
NIMG = NB * T              # 72
NPIX = 256
NPAIR = NIMG // 2          # 36
ROWS = [4, 5, 6, 7, 8, 0, 1, 2, 3]
ALU = mybir.AluOpType
ACTF = mybir.ActivationFunctionType

SA = 16.0                       # residual-stream activation scale
FP8_RB = (True, True)           # per-resblock fp8 switch
POOLSCALE = 1.0 / (NPIX * SA)


def build_kernel():
    nc = bacc.Bacc("TRN2", target_bir_lowering=False, debug=False,
                   num_devices=NCORES)

    def din(name, shape, dt):
        return nc.dram_tensor(name, shape, dt, kind="ExternalInput")

    def cdt(r):
        return DT.float8e4 if FP8_RB[r] else DT.bfloat16

    xcol_d = din("xcol", [NPAIR, 128, 512], DT.bfloat16)
    w1T_d = din("w1T", [128, DIM], DT.bfloat16)              # x SA
    # conv weights bundled per-rb into single fp8 DMAs (order: r1|w2|r3[,id])
    wc_d = [din(f"wc_{r}", [128, 6656 + (512 if r == 0 else 0)], cdt(r))
            for r in range(R)]
    bias_d = din("bias", [128, 24], DT.float32)  # encb|b1|b2|b3|b3p (x SA)
    gw_d = din("gw", [128, 9728], DT.bfloat16)   # gihT|ghhT|wkT
    wkb_d = din("wkb", [128, K, 4], DT.float32)
    gs_d = din("gs", [NB, 2696], DT.float32)     # gbih|gbhh|gbc|mask|iota|id8
    id128bf_d = din("id128bf", [128, 128], DT.bfloat16)

    out_d = nc.dram_tensor("out", [NB, 2 * K], DT.float32, kind="ExternalOutput")

    zin_b = nc.dram_tensor("zin_b", [128, 4, K * NB], DT.bfloat16)
    zout_b = nc.dram_tensor("zout_b", [NCORES, 128, 4, K * NB], DT.bfloat16,
                            addr_space="Shared")

    from contextlib import ExitStack
    with tile.TileContext(nc) as tc, ExitStack() as stack:
        wp = stack.enter_context(tc.tile_pool(name="weights", bufs=1))
        persist = stack.enter_context(tc.tile_pool(name="persist", bufs=1))
        y1p_pool = stack.enter_context(tc.tile_pool(name="y1p", bufs=3))
        xcp = stack.enter_context(tc.tile_pool(name="xc", bufs=6))
        hp = stack.enter_context(tc.tile_pool(name="h", bufs=6))
        y2p = stack.enter_context(tc.tile_pool(name="y2", bufs=3))
        psp = stack.enter_context(tc.tile_pool(name="psum", bufs=8, space="PSUM"))
        sp = stack.enter_context(tc.tile_pool(name="small", bufs=2))
        scrp = stack.enter_context(tc.tile_pool(name="scr", bufs=4))

        def wtile(dram, shape, dt, q=None):
            t = wp.tile(shape, dt, tag=dram.name, name=f"w_{dram.name}")
            (q or nc.scalar).dma_start(t[:], dram[:])
            return t

        w1T = wtile(w1T_d, [128, DIM], DT.bfloat16)
        biasb = wtile(bias_d, [128, 24], DT.float32)
        encb = biasb[:, 0:4]
        b1 = biasb[:, 4:8].rearrange("p (r m) -> p r m", r=R)
        b2 = biasb[:, 8:12].rearrange("p (r m) -> p r m", r=R)
        b3 = biasb[:, 12:20].rearrange("p (r m) -> p r m", r=R)
        b3p = biasb[:, 20:24]
        r1w, w2w, r3w = [None] * R, [None] * R, [None] * R
        id128dr = None
        for r in range(R):  # rb0 weights land before rb1's
            wcb = wtile(wc_d[r], [128, 6656 + (512 if r == 0 else 0)], cdt(r))
            r1w[r] = wcb[:, 0:1024].rearrange("p (c m k) -> p c m k", c=4, m=2)
            w2w[r] = wcb[:, 1024:5632].rearrange("p (t m j k) -> p t m j k",
                                                 t=9, m=2, j=2)
            r3w[r] = wcb[:, 5632:6656].rearrange("p (m j k) -> p m j k",
                                                 m=4, j=2)
            if r == 0:
                id128dr = wcb[:, 6656:7168].rearrange(
                    "p (v j k) -> p v j k", v=2, j=2)
        gwb = wtile(gw_d, [128, 9728], DT.bfloat16, q=nc.gpsimd)
        gihT = gwb[:, 0:3072].rearrange("p (c h) -> p c h", c=4)
        ghhT = gwb[:, 3072:4608].rearrange("p (c h) -> p c h", c=2)
        wkT = gwb[:, 4608:9728].rearrange("p (k c d) -> p k c d", k=K, c=2)
        wkb = wtile(wkb_d, [128, K, 4], DT.float32, q=nc.gpsimd)
        gsb = wtile(gs_d, [NB, 2696], DT.float32, q=nc.gpsimd)
        gbih = gsb[:, 0:768]
        gbhh = gsb[:, 768:1536]
        gbc = gsb[:, 1536:2048]
        mask = gsb[:, 2048:2368]
        iota320 = gsb[:, 2368:2688]
        ident = gsb[:, 2688:2696]
        id128bf = wtile(id128bf_d, [128, 128], DT.bfloat16)

        zbuf = persist.tile([128, 4, NIMG], DT.bfloat16, tag="zbuf")
        hT = persist.tile([128, 2, NB], DT.bfloat16, tag="hT")
        out_sb = persist.tile([NB, 2 * K], DT.float32, tag="out_sb")
        y1pads = {}
        for r in range(R):
            y1pads[r] = [y1p_pool.tile([128, 2, 18, 40], cdt(r), tag=f"y1pad{r}",
                                       name=f"y1pad{r}_{i}") for i in range(3)]
            for ypad in y1pads[r]:
                nc.vector.memset(ypad[:], 0.0)

        # ---------------- conv encoder stages for one image pair -----------
        def stage_A(p, st):
            """conv1: 4 bf16 MMs -> h (dtype per rb0)."""
            xc = xcp.tile([128, 512], DT.bfloat16, tag="xc", name=f"xc{p}")
            nc.sync.dma_start(xc[:], xcol_d[p])
            h = hp.tile([128, 4, 512], cdt(0), tag="h0", name=f"h0_{p}")
            for m in range(4):
                ps = psp.tile([128, 512], DT.float32, tag="ps", name=f"Aps{p}{m}")
                nc.tensor.matmul(ps[:], w1T[:, m * 128:(m + 1) * 128], xc[:],
                                 start=True, stop=True)
                if m % 2 == 0:
                    nc.scalar.activation(h[:, m], ps[:], ACTF.Relu,
                                         bias=encb[:, m:m + 1])
                else:
                    nc.vector.tensor_scalar(h[:, m], ps[:], encb[:, m:m + 1],
                                            0.0, ALU.add, ALU.max)
            st['h'] = h

        def stage_B(p, r, st):
            """rb_a 1x1 512->256 -> y1pad planes."""
            h = st['h']
            y1p = y1pads[r][p % 3]
            for m in range(2):
                ps = psp.tile([128, 512], DT.float32, tag="ps", name=f"B{p}{r}{m}")
                if FP8_RB[r]:
                    for cp in range(2):
                        nc.tensor.matmul(ps[:], r1w[r][:, 2 * cp:2 * cp + 2, m, :]
                                         .rearrange("p c m -> p c m"),
                                         h[:, 2 * cp:2 * cp + 2, :],
                                         start=(cp == 0), stop=(cp == 1),
                                         perf_mode=DRMODE)
                else:
                    for c in range(4):
                        nc.tensor.matmul(ps[:], r1w[r][:, c, m, :], h[:, c],
                                         start=(c == 0), stop=(c == 3))
                dst = y1p[:, m, 1:17, 2:34]
                src = ps[:].rearrange("p (r c) -> p r c", r=16)
                if m == 0:
                    nc.scalar.activation(dst, src, ACTF.Relu,
                                         bias=b1[:, r, m:m + 1])
                else:
                    nc.vector.tensor_scalar(dst, src, b1[:, r, m:m + 1],
                                            0.0, ALU.add, ALU.max)
            st['y1p'] = y1p

        def stage_C(p, r, st):
            """rb_b 3x3 256->256 -> y2."""
            y1p = st['y1p']
            y2 = y2p.tile([128, 2, 512], cdt(r), tag="y2", name=f"y2_{p}{r}")
            for m in range(2):
                ps = psp.tile([128, 512], DT.float32, tag="ps", name=f"C{p}{r}{m}")
                if FP8_RB[r]:
                    for tap in range(9):
                        ky, kx = divmod(tap, 3)
                        nc.tensor.matmul(
                            ps[:].rearrange("p (r c) -> p r c", r=16),
                            w2w[r][:, tap, m, :, :],
                            y1p[:, :, ky:ky + 16, 2 * kx:2 * kx + 32],
                            start=(tap == 0), stop=(tap == 8),
                            perf_mode=DRMODE)
                else:
                    i_mm = 0
                    for c in range(2):
                        for tap in range(9):
                            ky, kx = divmod(tap, 3)
                            nc.tensor.matmul(
                                ps[:].rearrange("p (r c) -> p r c", r=16),
                                w2w[r][:, tap, m, c, :],
                                y1p[:, c, ky:ky + 16, 2 * kx:2 * kx + 32],
                                start=(i_mm == 0), stop=(i_mm == 17))
                            i_mm += 1
                if m == 0:
                    nc.scalar.activation(y2[:, m], ps[:], ACTF.Relu,
                                         bias=b2[:, r, m:m + 1])
                else:
                    nc.vector.tensor_scalar(y2[:, m], ps[:], b2[:, r, m:m + 1],
                                            0.0, ALU.add, ALU.max)
            st['y2'] = y2

        def stage_D(p, r, st):
            """rb_c 1x1 256->512 + residual (+ pool on last rb)."""
            y2 = st['y2']
            resid = st['h']
            is_last = (r == R - 1)
            ps3 = [psp.tile([128, 512], DT.float32, tag="ps",
                            name=f"D{p}{r}{m}") for m in range(4)]
            for m in range(4):
                if FP8_RB[r]:
                    nc.tensor.matmul(ps3[m][:], r3w[r][:, m, :, :], y2[:],
                                     start=True, stop=False, perf_mode=DRMODE)
                    # residual via DR identity: pair (m, m^1) chunks of h
                    # with weights (I, 0) / (0, I) -> psum += h[:, m]
                    mlo = m & ~1
                    nc.tensor.matmul(ps3[m][:], id128dr[:, m & 1, :, :],
                                     resid[:, mlo:mlo + 2, :],
                                     start=False, stop=True, perf_mode=DRMODE)
                else:
                    for c in range(2):
                        nc.tensor.matmul(ps3[m][:], r3w[r][:, m, c, :], y2[:, c],
                                         start=(c == 0), stop=False)
                    nc.tensor.matmul(ps3[m][:], id128bf[:], resid[:, m],
                                     start=False, stop=True)
            if not is_last:
                hn = hp.tile([128, 4, 512], cdt(r + 1), tag="h1",
                             name=f"h1_{p}")
                for m in range(4):
                    if m % 2 == 0:
                        nc.scalar.activation(hn[:, m], ps3[m][:], ACTF.Relu,
                                             bias=b3[:, r, m:m + 1])
                    else:
                        nc.vector.tensor_scalar(hn[:, m], ps3[m][:],
                                                b3[:, r, m:m + 1], 0.0,
                                                ALU.add, ALU.max)
            else:
                # relu + global-avg-pool fused on ACT (relu(s*psum + s*b3),
                # accum_out sums the free dim); pooled scalars land in zt
                t_idx, j_idx = divmod(p, 4)
                col = t_idx * 8 + 2 * j_idx
                zt = sp.tile([128, 4, 2], DT.float32, tag="zt", name=f"zt{p}")
                for m in range(4):
                    tv = ps3[m][:].rearrange("p (x i) -> p i x", i=2)
                    for i in range(2):
                        scr = scrp.tile([128, 256], DT.float8e4,
                                        tag=f"pscr{m}{i}",
                                        name=f"pscr{p}{m}{i}")
                        nc.scalar.activation(scr[:], tv[:, i], ACTF.Relu,
                                             bias=b3p[:, m:m + 1],
                                             scale=POOLSCALE,
                                             accum_out=zt[:, m, i:i + 1])
                nc.vector.tensor_copy(zbuf[:, :, col:col + 2], zt[:])
            if not is_last:
                st['h'] = hn

        # ---------------- GRU step ----------------
        gru_state = {'h': None}

        def emit_gru_step(t):
            ve = nc.vector
            gi_rz = psp.tile([NB, 2 * HID], DT.float32, tag="ps", name=f"girz{t}")
            gi_n = psp.tile([NB, HID], DT.float32, tag="ps", name=f"gin{t}")
            for c in range(4):
                nc.tensor.matmul(gi_rz[:], zbuf[:, c, t * 8:(t + 1) * 8],
                                 gihT[:, c, :2 * HID],
                                 start=(c == 0), stop=(c == 3))
            for c in range(4):
                nc.tensor.matmul(gi_n[:], zbuf[:, c, t * 8:(t + 1) * 8],
                                 gihT[:, c, 2 * HID:],
                                 start=(c == 0), stop=(c == 3))
            gh_rz = gh_n = None
            if t > 0:
                gh_rz = psp.tile([NB, 2 * HID], DT.float32, tag="ps",
                                 name=f"ghrz{t}")
                gh_n = psp.tile([NB, HID], DT.float32, tag="ps", name=f"ghn{t}")
                for c in range(2):
                    nc.tensor.matmul(gh_rz[:], hT[:, c, :], ghhT[:, c, :2 * HID],
                                     start=(c == 0), stop=(c == 1))
                for c in range(2):
                    nc.tensor.matmul(gh_n[:], hT[:, c, :], ghhT[:, c, 2 * HID:],
                                     start=(c == 0), stop=(c == 1))

            rz = sp.tile([NB, 2 * HID], DT.float32, tag="rz", name=f"rz{t}")
            ng = sp.tile([NB, HID], DT.float32, tag="ng", name=f"ng{t}")
            tmp = sp.tile([NB, HID], DT.float32, tag="gtmp", name=f"gtmp{t}")
            ve.tensor_add(rz[:], gi_rz[:], gbc[:])
            if t > 0:
                ve.tensor_add(rz[:], rz[:], gh_rz[:])
            nc.scalar.activation(rz[:], rz[:], ACTF.Sigmoid)
            if t > 0:
                ve.tensor_add(tmp[:], gh_n[:], gbhh[:, 2 * HID:])
            else:
                ve.tensor_copy(tmp[:], gbhh[:, 2 * HID:])
            ve.tensor_mul(tmp[:], tmp[:], rz[:, :HID])
            ve.tensor_add(ng[:], gi_n[:], gbih[:, 2 * HID:])
            ve.tensor_add(ng[:], ng[:], tmp[:])
            nc.scalar.activation(ng[:], ng[:], ACTF.Tanh)
            h_new = sp.tile([NB, HID], DT.float32, tag=f"hstep{t}",
                            name=f"hnew{t}")
            if t == 0:
                ve.tensor_mul(tmp[:], rz[:, HID:], ng[:])
                ve.tensor_sub(h_new[:], ng[:], tmp[:])
            else:
                ve.tensor_sub(tmp[:], gru_state['h'][:], ng[:])
                ve.tensor_mul(tmp[:], rz[:, HID:], tmp[:])
                ve.tensor_add(h_new[:], ng[:], tmp[:])
            gru_state['h'] = h_new

        def emit_transposes(t):
            h_new = gru_state['h']
            for c in range(2):
                pt = psp.tile([128, NB], DT.float32, tag="ps", name=f"pt{t}{c}")
                nc.tensor.transpose(pt[:], h_new[:, c * 128:(c + 1) * 128],
                                    ident)
                nc.vector.tensor_copy(hT[:, c, :], pt[:])

        # -------- emit: rows x pair-groups, stage-interleaved --------
        after_group = {0: [], 1: []}
        ztk = persist.tile([128, NCORES, 4, K * NB], DT.bfloat16, tag="ztk")
        for t in ROWS:
            for g in range(2):
                p0, p1 = t * 4 + 2 * g, t * 4 + 2 * g + 1
                st0, st1 = {}, {}
                stage_A(p0, st0)
                stage_A(p1, st1)
                for r in range(R):
                    stage_B(p0, r, st0)
                    stage_B(p1, r, st1)
                    stage_C(p0, r, st0)
                    stage_C(p1, r, st1)
                    stage_D(p0, r, st0)
                    stage_D(p1, r, st1)
                if after_group[g]:
                    for fn in after_group[g]:
                        fn()
                    after_group[g] = []
            if t == 8:
                nc.gpsimd.dma_start(zin_b.ap(), zbuf[:, :, TCTX * 8:])
                nc.gpsimd.collective_compute(
                    "AllGather", ALU.bypass,
                    replica_groups=[list(range(NCORES))],
                    ins=[zin_b.ap().opt()], outs=[zout_b.ap().opt()])
                for core in range(NCORES):
                    nc.gpsimd.dma_start(ztk[:, core], zout_b[core])
            if t < 3:
                after_group[0].append(lambda t=t: emit_gru_step(t))
                after_group[1].append(lambda t=t: emit_transposes(t))
            elif t == 3:
                emit_gru_step(3)
                emit_transposes(3)

        # ---------------- preds + scores + batched log-softmax ------------
        preds = persist.tile([128, 4, K * NB], DT.bfloat16, tag="preds")
        for k in range(K):
            for m in range(4):
                pp = psp.tile([128, NB], DT.float32, tag="ps", name=f"pp{k}{m}")
                for c in range(2):
                    nc.tensor.matmul(pp[:], wkT[:, k, c, m * 128:(m + 1) * 128],
                                     hT[:, c, :], start=(c == 0), stop=(c == 1))
                nc.scalar.activation(preds[:, m, k * NB:(k + 1) * NB], pp[:],
                                     ACTF.Identity, bias=wkb[:, k, m:m + 1])

        y40 = persist.tile([NB, K * B], DT.float32, tag="y40")
        ek40 = persist.tile([NB, K * B], DT.float32, tag="ek40")
        eq40 = persist.tile([NB, K * B], DT.float32, tag="eq40")
        mx = sp.tile([NB, K], DT.float32, tag="mx")
        nmx = sp.tile([NB, K], DT.float32, tag="nmx")
        sek = sp.tile([NB, K], DT.float32, tag="sek")
        dg = sp.tile([NB, K], DT.float32, tag="dg")
        lg = sp.tile([NB, K], DT.float32, tag="lg")
        for k in range(K):
            psk = psp.tile([NB, B], DT.float32, tag="ps", name=f"sck{k}")
            for c in range(4):
                nc.tensor.matmul(psk[:], preds[:, c, k * NB:(k + 1) * NB],
                                 ztk[:, :, c, k * NB:(k + 1) * NB],
                                 start=(c == 0), stop=(c == 3))
            nc.scalar.activation(y40[:, k * B:(k + 1) * B], psk[:], ACTF.Exp)
        y3d = y40[:].rearrange("p (k b) -> p k b", k=K)
        nc.vector.tensor_reduce(mx[:], y3d, mybir.AxisListType.X, ALU.max)
        nc.vector.tensor_scalar_mul(nmx[:], mx[:], -1.0)
        for k in range(K):
            nc.scalar.activation(ek40[:, k * B:(k + 1) * B],
                                 y40[:, k * B:(k + 1) * B], ACTF.Exp,
                                 bias=nmx[:, k:k + 1],
                                 accum_out=sek[:, k:k + 1])
            nc.vector.tensor_scalar(eq40[:, k * B:(k + 1) * B],
                                    y40[:, k * B:(k + 1) * B], mx[:, k:k + 1],
                                    0.0, ALU.subtract, ALU.is_equal)
        nc.vector.tensor_mul(ek40[:], y40[:], mask)
        nc.vector.tensor_reduce(dg[:], ek40[:].rearrange("p (k b) -> p k b", k=K),
                                mybir.AxisListType.X, ALU.add)
        nc.vector.tensor_mul(eq40[:], eq40[:], iota320)
        nc.vector.tensor_reduce(out_sb[:, K:], eq40[:].rearrange(
            "p (k b) -> p k b", k=K), mybir.AxisListType.X, ALU.add)
        nc.scalar.activation(lg[:], sek[:], ACTF.Ln)
        nc.vector.tensor_add(lg[:], lg[:], mx[:])
        nc.vector.tensor_sub(out_sb[:, :K], dg[:], lg[:])

        nc.sync.dma_start(out_d[:], out_sb[:])

    nc.compile()
    return nc


ED_ROUND = True          # error-diffusion fp8 weight rounding
ED_MEAN_IMGS = 24        # images sampled for layer-input channel means


def _conv_mm(x, w, b, stride=1, pad=0):
    N, Ci, H, W = x.shape
    O, _, kh, kw = w.shape
    if pad:
        x = np.pad(x, ((0, 0), (0, 0), (pad, pad), (pad, pad)))
    Ho = (H + 2 * pad - kh) // stride + 1
    Wo = (W + 2 * pad - kw) // stride + 1
    s = x.strides
    win = np.lib.stride_tricks.as_strided(
        x, (N, Ci, kh, kw, Ho, Wo),
        (s[0], s[1], s[2], s[3], stride * s[2], stride * s[3]))
    col = np.ascontiguousarray(win).reshape(N, Ci * kh * kw, Ho * Wo)
    wm = w.reshape(O, Ci * kh * kw)
    out = np.empty((N, O, Ho * Wo), F32)
    for i in range(N):
        out[i] = wm @ col[i]
    return out.reshape(N, O, Ho, Wo) + b[None, :, None, None]


def _collect_means(inputs, n_img):
    """Per-conv-layer input channel means from a subsample of images."""
    x = np.asarray(inputs['x'], F32).reshape(-1, C, 32, 32)[:n_img]
    h = np.maximum(_conv_mm(x, np.asarray(inputs['enc_w'], F32),
                            np.asarray(inputs['enc_b'], F32), 2, 2), 0)
    means = []
    for i in range(R):
        r = h
        means.append(h.mean(axis=(0, 2, 3)))
        y = np.maximum(_conv_mm(h, np.asarray(inputs['res_w1'][i], F32),
                                np.asarray(inputs['res_b1'][i], F32)), 0)
        means.append(y.mean(axis=(0, 2, 3)))
        y = np.maximum(_conv_mm(y, np.asarray(inputs['res_w2'][i], F32),
                                np.asarray(inputs['res_b2'][i], F32), pad=1), 0)
        means.append(y.mean(axis=(0, 2, 3)))
        y = _conv_mm(y, np.asarray(inputs['res_w3'][i], F32),
                     np.asarray(inputs['res_b3'][i], F32))
        h = np.maximum(y + r, 0)
    return means


def _ed_round_w(wt, mu):
    """Error-diffusion fp8 rounding: per output row, keeps the mu-weighted
    rounding-error sum near zero (the component that survives avg-pooling)."""
    O = wt.shape[0]
    Cin = wt.shape[1]
    taps = int(np.prod(wt.shape[2:])) if wt.ndim > 2 else 1
    w2 = wt.reshape(O, Cin * taps).astype(F32)
    muf = np.repeat(mu.astype(F32), taps)
    q = np.empty_like(w2)
    carry = np.zeros(O, F32)
    for k in range(Cin * taps):
        m = muf[k]
        if abs(m) > 1e-6:
            lim = np.abs(w2[:, k]) * 0.25 + 1e-4
            t = w2[:, k] - np.clip(carry / m, -lim, lim)
        else:
            t = w2[:, k]
        qk = t.astype(E4).astype(F32)
        q[:, k] = qk
        carry += (qk - w2[:, k]) * m
    return q.reshape(wt.shape)


def host_prep(inputs):
    """Host-side prep: im2col (image-interleaved), weight packing."""
    x = np.asarray(inputs['x'], F32)
    xp = np.pad(x, ((0, 0), (0, 0), (0, 0), (2, 2), (2, 2)))
    s = xp.strides
    xs = np.lib.stride_tricks.as_strided(
        xp, shape=(B, T, C, 5, 5, 16, 16),
        strides=(s[0], s[1], s[2], s[3], s[4], 2 * s[3], 2 * s[4]))
    x_col = np.ascontiguousarray(xs).reshape(B, T, 75, 16, 16).astype(BF16)

    xcols = []
    for core in range(NCORES):
        xc = x_col[core * NB:(core + 1) * NB]
        arr = np.zeros((NPAIR, 128, 512), BF16)
        av = arr[:, :75].reshape(NPAIR, 75, 16, 16, 2)
        for t in range(T):
            for j in range(NB // 2):
                p = t * 4 + j
                av[p, :, :, :, 0] = xc[2 * j, t]
                av[p, :, :, :, 1] = xc[2 * j + 1, t]
        xcols.append(arr)

    def pdt(r):
        return E4 if FP8_RB[r] else BF16

    w = {}
    w1T = np.zeros((128, DIM), F32)
    w1T[:75] = np.asarray(inputs['enc_w'], F32).reshape(DIM, 75).T * SA
    w['w1T'] = w1T.astype(BF16)

    rw = {n: np.asarray(inputs[n], F32).copy()
          for n in ('res_w1', 'res_w2', 'res_w3')}
    if ED_ROUND and any(FP8_RB):
        means = _collect_means(inputs, ED_MEAN_IMGS)
        for r in range(R):
            if not FP8_RB[r]:
                continue
            for j, n in enumerate(('res_w1', 'res_w2', 'res_w3')):
                rw[n][r] = _ed_round_w(rw[n][r], means[3 * r + j])
    r1 = rw['res_w1'].reshape(R, 2, 128, 4, 128)
    r2 = rw['res_w2'].reshape(R, 2, 128, 2, 128, 3, 3)
    r3 = rw['res_w3'].reshape(R, 4, 128, 2, 128)
    idr = np.zeros((128, 2, 2, 128), F32)
    idr[:, 0, 0] = np.eye(128, dtype=F32)
    idr[:, 1, 1] = np.eye(128, dtype=F32)
    for r in range(R):
        a = np.ascontiguousarray(r1[r].transpose(3, 2, 0, 1)).reshape(128, -1)
        b = np.ascontiguousarray(
            r2[r].transpose(3, 4, 5, 0, 2, 1)).reshape(128, -1)
        c = np.ascontiguousarray(r3[r].transpose(3, 0, 2, 1)).reshape(128, -1)
        parts = [a, b, c] + ([idr.reshape(128, -1)] if r == 0 else [])
        w[f'wc_{r}'] = np.concatenate(parts, axis=1).astype(pdt(r))
    encb_h = np.ascontiguousarray(
        np.asarray(inputs['enc_b'], F32).reshape(4, 128).T) * SA
    b1_h = np.ascontiguousarray(
        np.asarray(inputs['res_b1'], F32).reshape(R, 2, 128).transpose(2, 0, 1)) * SA
    b2_h = np.ascontiguousarray(
        np.asarray(inputs['res_b2'], F32).reshape(R, 2, 128).transpose(2, 0, 1)) * SA
    b3_h = np.ascontiguousarray(
        np.asarray(inputs['res_b3'], F32).reshape(R, 4, 128).transpose(2, 0, 1)) * SA
    b3p_h = np.ascontiguousarray(
        np.asarray(inputs['res_b3'], F32)[R - 1].reshape(4, 128).T) * SA * POOLSCALE
    w['bias'] = np.concatenate(
        [encb_h, b1_h.reshape(128, -1), b2_h.reshape(128, -1),
         b3_h.reshape(128, -1), b3p_h], axis=1).astype(F32)

    gihT_h = np.ascontiguousarray(
        np.asarray(inputs['gru_w_ih'], F32).T.reshape(4, 128, 3 * HID)
        .transpose(1, 0, 2)).reshape(128, -1)
    ghhT_h = np.ascontiguousarray(
        np.asarray(inputs['gru_w_hh'], F32).T.reshape(2, 128, 3 * HID)
        .transpose(1, 0, 2)).reshape(128, -1)
    wk = np.asarray(inputs['wk_w'], F32).transpose(0, 2, 1)
    wkT_h = np.ascontiguousarray(
        wk.reshape(K, 2, 128, DIM).transpose(2, 0, 1, 3)).reshape(128, -1)
    w['gw'] = np.concatenate([gihT_h, ghhT_h, wkT_h], axis=1).astype(BF16)
    w['wkb'] = np.ascontiguousarray(
        np.asarray(inputs['wk_b'], F32).reshape(K, 4, 128).transpose(2, 0, 1))
    bih = np.asarray(inputs['gru_b_ih'], F32)
    bhh = np.asarray(inputs['gru_b_hh'], F32)
    gs = np.concatenate(
        [np.tile(bih[None, :], (NB, 1)),
         np.tile(bhh[None, :], (NB, 1)),
         np.tile((bih + bhh)[None, :2 * HID], (NB, 1)),
         np.zeros((NB, K * B), F32),          # mask placeholder (per core)
         np.tile(np.arange(B, dtype=F32)[None, :], (NB, K)),
         np.eye(NB, dtype=F32)], axis=1).astype(F32)
    w['gs'] = gs
    w['id128bf'] = np.eye(128, dtype=F32).astype(BF16)
    return xcols, w


_NC_CACHE = {}


def get_nc():
    if 'nc' not in _NC_CACHE:
        _NC_CACHE['nc'] = build_kernel()
    return _NC_CACHE['nc']


def make_in_maps(inputs):
    xcols, w = host_prep(inputs)
    in_maps = []
    for core in range(NCORES):
        m = dict(w)
        m['xcol'] = xcols[core]
        msk = np.zeros((NB, K, B), F32)
        for i in range(NB):
            msk[i, :, core * NB + i] = 1.0
        gs = w['gs'].copy()
        gs[:, 2048:2368] = msk.reshape(NB, K * B)
        m['gs'] = gs
        in_maps.append(m)
    return in_maps


def reduce_outputs(results):
    tot, correct = 0.0, 0
    for core in range(NCORES):
        o = np.asarray(results[core]['out'], F32)
        tot += float(o[:, :K].sum())
        for i in range(NB):
            correct += int((o[i, K:] == core * NB + i).sum())
    loss = np.float32(-tot / (B * K))
    acc = np.float32(correct / (B * K))
    return loss, acc


def _install_ntff_hook():
    try:
        from antenv.axon_hooks import get_axon_ntff_profile_hook  # noqa: F401
        return
    except ImportError:
        pass
    import ctypes
    import types
    import contextlib

    so_path = "/opt/axon/libaxon_pjrt.so"
    if not os.path.exists(so_path):
        return
    lib = ctypes.CDLL(so_path)
    if not hasattr(lib, "axon_start_nrt_profile"):
        return
    lib.axon_start_nrt_profile.argtypes = [ctypes.POINTER(ctypes.c_int64),
                                           ctypes.c_size_t]
    lib.axon_start_nrt_profile.restype = ctypes.c_int64
    lib.axon_stop_nrt_profile.argtypes = [ctypes.c_char_p]
    lib.axon_stop_nrt_profile.restype = ctypes.c_int64

    @contextlib.contextmanager
    def _hook(output_dir, device_ids):
        import jax
        jax.devices()
        if device_ids:
            ids = (ctypes.c_int64 * len(device_ids))(*device_ids)
            rc = lib.axon_start_nrt_profile(ids, len(device_ids))
        else:
            rc = lib.axon_start_nrt_profile(None, 0)
        if rc != 0:
            raise RuntimeError(f"axon_start_nrt_profile rc={rc}")
        try:
            yield
        finally:
            n = lib.axon_stop_nrt_profile(str(output_dir).encode())
            print(f"ntff profile: {n} file(s) written to {output_dir}")

    mod = types.ModuleType("antenv.axon_hooks")
    mod.get_axon_ntff_profile_hook = lambda: _hook
    mod.set_axon_ntff_profile_hook = lambda h: None
    import antenv
    antenv.axon_hooks = mod
    sys.modules["antenv.axon_hooks"] = mod


def run(inputs, trace=False, **kw):
    if trace:
        _install_ntff_hook()
    nc = get_nc()
    in_maps = make_in_maps(inputs)
    res = run_bass_kernel_spmd(nc, in_maps, core_ids=list(range(NCORES)),
                               trace=trace, **kw)
    return res


def kernel(**inputs):
    res = run(inputs, trace=False)
    return reduce_outputs(res.results)


if __name__ == '__main__':
    import reference as Rf
    inputs = {k: np.asarray(v) for k, v in Rf.setup_inputs().items()}
    loss, acc = kernel(**inputs)
    print('kernel loss/acc:', loss, acc)
